# revision 90
# baseline (speedup 1.0000x reference)
"""Transformer-XL attention on 8 Trainium2 NeuronCores (Bass/Tile).

Sharding: 8 cores = 4 batches x 2 head-groups of 8 heads.
Each core computes its (batch, head-group) attention output projected through
its W_proj row-slice; host sums the two head-group partials per batch and adds
the bias terms (b_v @ W_proj + b_proj) once.

v2 dataflow (vs v1): position logits are merged into the content logits in
PSUM via an identity-matmul accumulate on the PE, so only ONE exp per score
element runs on the Act engine (v1 ran two).  The rel-shift skew DMA stages
through a DRAM scratch ring; PSUM->SBUF conversion of position logits rides
on the DVE (last stripe on Act).  1/Z comes from a reciprocal read straight
off the PSUM z-row; per-head norms are emitted one head late so Pool's
in-order queue never blocks the next head's stripe pipeline; the output
projection is software-pipelined so its ct0-2 partials overlap the final
norm chain.

Self-contained: only imports numpy/ml_dtypes and the installed concourse stack.
"""

import sys

for _p in ("/opt/trn_rl_repo",):
    if _p not in sys.path:
        sys.path.insert(0, _p)

from contextlib import ExitStack

import ml_dtypes
import numpy as np

import concourse.bacc as bacc
import concourse.bass as bass
import concourse.mybir as mybir
import concourse.tile as tile
from concourse.bass_utils import run_bass_kernel_spmd

CUR, FULL, BS, D = 1024, 2048, 4, 1024
HN, HD = 16, 64
PREV = FULL - CUR
SCALE = 1.0 / HD**0.5
HC = 8          # heads per core
CW = HC * HD    # 512 channel columns per core
BF = mybir.dt.bfloat16
F32 = mybir.dt.float32
EXP = mybir.ActivationFunctionType.Exp
BF_NP = ml_dtypes.bfloat16
NEG = -1.0e9    # logit pad for masked region

_CACHE = {}


def _ap(t, off, dims):
    return bass.AP(tensor=t.tensor, offset=t.offset + off, ap=dims)


def _blk(d, rowlen, nblk):
    """DRAM [nblk*128, rowlen] viewed as [p, blk, col]."""
    return _ap(d, 0, [[rowlen, 128], [128 * rowlen, nblk], [1, rowlen]])


def build_program():
    nc = bacc.Bacc("TRN2", target_bir_lowering=False, debug=False)

    XcT = nc.dram_tensor("XcT", [D, CUR], BF, kind="ExternalInput").ap()
    XfT = nc.dram_tensor("XfT", [D, FULL], BF, kind="ExternalInput").ap()
    PosT = nc.dram_tensor("PosT", [D, FULL], BF, kind="ExternalInput").ap()
    Wq = nc.dram_tensor("Wq", [D, CW], BF, kind="ExternalInput").ap()
    Wk = nc.dram_tensor("Wk", [D, CW], BF, kind="ExternalInput").ap()
    Wv = nc.dram_tensor("Wv", [D, CW], BF, kind="ExternalInput").ap()
    Wpos = nc.dram_tensor("Wpos", [D, CW], BF, kind="ExternalInput").ap()
    Wproj = nc.dram_tensor("Wproj", [CW, D], BF, kind="ExternalInput").ap()
    Ident = nc.dram_tensor("Ident", [128, 128], BF, kind="ExternalInput").ap()
    qu_b_d = nc.dram_tensor("qu_b", [CW, 1], F32, kind="ExternalInput").ap()
    qv_b_d = nc.dram_tensor("qv_b", [CW, 1], F32, kind="ExternalInput").ap()
    k_b_d = nc.dram_tensor("k_b", [CW, 1], F32, kind="ExternalInput").ap()
    r_b_d = nc.dram_tensor("r_b", [CW, 1], F32, kind="ExternalInput").ap()
    out_d = nc.dram_tensor("out_part", [CUR, D], BF, kind="ExternalOutput").ap()
    z_dram = nc.dram_tensor("z_scratch", [HC, CUR], F32).ap()
    gs_dram = nc.dram_tensor("gs_scratch", [8 * 128, 2048], BF).ap()

    with tile.TileContext(nc) as tc, ExitStack() as ctx:
        persist = ctx.enter_context(tc.tile_pool(name="persist", bufs=1))
        ps_pool = ctx.enter_context(tc.tile_pool(name="ps", bufs=3, space="PSUM"))
        gp_pool = ctx.enter_context(tc.tile_pool(name="gp", bufs=3, space="PSUM"))
        av_pool = ctx.enter_context(tc.tile_pool(name="avps", bufs=2, space="PSUM"))
        gtpool = ctx.enter_context(tc.tile_pool(name="gt", bufs=2))
        gpool = ctx.enter_context(tc.tile_pool(name="g", bufs=3))
        epool = ctx.enter_context(tc.tile_pool(name="e", bufs=4))
        zrpool = ctx.enter_context(tc.tile_pool(name="zr", bufs=1))
        stpool = ctx.enter_context(tc.tile_pool(name="st", bufs=2))
        obpool = ctx.enter_context(tc.tile_pool(name="ob", bufs=3))
        wpool = ctx.enter_context(tc.tile_pool(name="wp", bufs=1))
        xpool = ctx.enter_context(tc.tile_pool(name="xp", bufs=3))

        QuT = persist.tile([128, 4 * CUR], BF, tag="QuT")
        QvT = persist.tile([128, 4 * CUR], BF, tag="QvT")
        KT = persist.tile([128, 4 * FULL], BF, tag="KT")
        RT = persist.tile([128, 4 * FULL], BF, tag="RT")
        Vp = persist.tile([128, 16 * 8 * 66], BF, tag="Vp")
        OT0 = persist.tile([128, CUR], BF, tag="OT0")
        OT1 = persist.tile([128, CUR], BF, tag="OT1")
        OT2 = persist.tile([128, CUR], BF, tag="OT2")
        OT3 = persist.tile([128, CUR], BF, tag="OT3")
        OTs = [OT0, OT1, OT2, OT3]
        Id = persist.tile([128, 128], BF, tag="Id")
        biases = persist.tile([128, 16], F32, tag="biases")
        CPY = mybir.ActivationFunctionType.Copy

        def load_small():
            for bi, bd in enumerate((qu_b_d, qv_b_d, k_b_d, r_b_d)):
                nc.sync.dma_start(
                    out=biases[:, bi * 4:(bi + 1) * 4],
                    in_=_ap(bd, 0, [[1, 128], [128, 4]]),
                )
            nc.sync.dma_start(out=Id, in_=Ident)
            # ones columns of V' (col 64 of each 66-wide head slot)
            nc.vector.memset(
                _ap(Vp, 64, [[16 * 8 * 66, 128], [8 * 66, 16], [66, 8], [1, 1]]), 1.0)

        # ---------------- streamed projections ----------------
        def load_w(tag, wd):
            w = wpool.tile([128, 8 * CW], BF, tag=tag)
            with tc.high_priority():
                nc.sync.dma_start(out=w.rearrange("p (kt c) -> p kt c", kt=8),
                                  in_=_blk(wd, CW, 8))
            return w

        def x_chunk(src, n_total, c):
            """256-token chunk c of DRAM [D, n_total] -> [128, 8kt x 256]."""
            xch = xpool.tile([128, 2048], BF, tag="xch")
            with tc.high_priority():
                nc.sync.dma_start(
                    out=xch.rearrange("p (k c) -> p k c", k=8),
                    in_=_ap(src, c * 256, [[n_total, 128], [128 * n_total, 8], [1, 256]]))
            return xch

        def proj_chunk(w_sb, xch, ct, dests):
            """dests: list of (out_256col_ap, bias_col)."""
            ps = ps_pool.tile([128, 512], F32, tag="ps")
            for kt in range(8):
                nc.tensor.matmul(
                    ps[:, 0:256],
                    w_sb[:, kt * CW + ct * 128: kt * CW + ct * 128 + 128],
                    xch[:, kt * 256: kt * 256 + 256],
                    start=(kt == 0), stop=(kt == 7),
                )
            for dest, bcol in dests:
                nc.vector.tensor_scalar(
                    dest, ps[:, 0:256],
                    biases[:, bcol * 4 + ct: bcol * 4 + ct + 1],
                    None, mybir.AluOpType.add,
                )

        def v_chunk(wv_sb, xch, tt):
            """V projection for 128-token block tt (tokens tt*128..+128)."""
            tts = tt % 2
            ps = ps_pool.tile([128, 512], F32, tag="ps")
            for kt in range(8):
                nc.tensor.matmul(
                    ps[:, 0:512],
                    xch[:, kt * 256 + tts * 128: kt * 256 + tts * 128 + 128],
                    wv_sb[:, kt * CW: kt * CW + CW],
                    start=(kt == 0), stop=(kt == 7),
                )
            nc.scalar.activation(
                _ap(Vp, tt * 8 * 66, [[16 * 8 * 66, 128], [66, 8], [1, 64]]),
                ps[:, 0:512].rearrange("p (h d) -> p h d", h=8),
                mybir.ActivationFunctionType.Copy,
            )

        # ---------------- attention helpers ----------------
        def gen_pos(h, GT):
            """Emit head h's position-logit stripes (one per query block qt).
            Each stripe: PE matmuls (512-col chunks) -> DVE convert to bf16
            Gl -> pad memset -> skew DMA -> DmaTransposeAnt into key-major GT.
            Generator yields after each 512-col CHUNK so the caller can
            interleave stripe work finely with other PE work (keeps the DVE
            conversion from falling behind the PSUM ring)."""
            ct = h // 2
            rb = (h % 2) * 64
            for qt in range(8):
                i0 = qt * 128
                m_lo = 896 - i0
                W = FULL - m_lo            # 1152 + i0 == Wj (multiple of 128)
                nblk = qt + 9
                Gl = gpool.tile([128, 2176], BF, tag="Gl")
                nc.gpsimd.memset(Gl[:, W:W + 128], NEG)
                off = 0
                while off < W:
                    wn = min(512, W - off)
                    gps = gp_pool.tile([128, 512], F32, tag="gp")
                    nc.tensor.matmul(
                        gps[:, 0:wn],
                        QvT[rb:rb + 64, ct * CUR + i0: ct * CUR + i0 + 128],
                        RT[rb:rb + 64, ct * FULL + m_lo + off:
                           ct * FULL + m_lo + off + wn],
                        start=True, stop=True,
                    )
                    if qt == 7:
                        # last stripe converts on Act: DVE is the convP
                        # pipeline's slowest stage; shed its backlog peak
                        nc.scalar.activation(Gl[:, off:off + wn], gps[:, 0:wn], CPY)
                    else:
                        nc.vector.tensor_copy(Gl[:, off:off + wn], gps[:, 0:wn])
                    off += wn
                    if off < W:
                        yield
                # skew: gs[p, j] = Gl[p, 127 + j - p]  (diagonal read) into a
                # DRAM scratch ring (deep buffering, frees SBUF)
                gs = _ap(gs_dram, ((h * 8 + qt) % 8) * 128 * 2048,
                         [[2048, 128], [1, W]])
                nc.sync.dma_start(out=gs, in_=_ap(Gl, 127, [[2176 - 1, 128], [1, W]]))
                # transpose into GT[jj, qt, t, p] = gs[p, t*128 + jj]
                nc.sync.dma_start_transpose(
                    out=_ap(GT, qt * 2048, [[16 * 8 * 128, 128], [128, nblk], [1, 128]]),
                    in_=gs,
                )
                yield

        def adv(gen, n):
            if gen is None:
                return
            for _ in range(n):
                next(gen, None)

        def score_part(h, t, GT):
            """content scores + position logits (identity-matmul accumulate)
            -> one exp on Act -> E(t).  AV is emitted LAG iterations later."""
            ct = h // 2
            rb = (h % 2) * 64
            qt_min = max(0, t - 8)
            ioff = qt_min * 128
            w = CUR - ioff
            E = epool.tile([128, 1024], BF, tag="E")
            sc = 0
            while sc < w:
                wn = min(512, w - sc)
                nqt = wn // 128
                cps = ps_pool.tile([128, 512], F32, tag="ps")
                nc.tensor.matmul(
                    cps[:, 0:wn],
                    KT[rb:rb + 64, ct * FULL + t * 128: ct * FULL + t * 128 + 128],
                    QuT[rb:rb + 64, ct * CUR + ioff + sc: ct * CUR + ioff + sc + wn],
                    start=True, stop=False,
                )
                nc.tensor.matmul(
                    cps[:, 0:wn],
                    Id,
                    _ap(GT, (qt_min + sc // 128) * 2048 + t * 128,
                        [[16 * 8 * 128, 128], [2048, nqt], [1, 128]]),
                    start=False, stop=True,
                )
                nc.scalar.activation(E[:, sc:sc + wn], cps[:, 0:wn], EXP, scale=SCALE)
                sc += wn
            return E

        def av_part(h, t, E, avs):
            ioff = max(0, t - 8) * 128
            for c in range(2):
                lo = max(ioff, c * 512)
                hi = (c + 1) * 512
                if lo >= hi:
                    continue
                last_t = 11 if c == 0 else 15
                nc.tensor.matmul(
                    avs[c][:, lo - c * 512: hi - c * 512],
                    Vp[:, t * 8 * 66 + h * 66: t * 8 * 66 + h * 66 + 65],
                    E[:, lo - ioff: hi - ioff],
                    start=(t == 0), stop=(t == last_t),
                )

        def evict_c(h, avs, c, zi):
            """OT eviction on Act + 1/Z straight off the PSUM z-row on DVE."""
            ct = h // 2
            rb = (h % 2) * 64
            nc.scalar.activation(
                OTs[ct][rb:rb + 64, c * 512: c * 512 + 512],
                avs[c][0:64, :], CPY,
            )
            nc.vector.reciprocal(zi[0:1, c * 512:(c + 1) * 512], avs[c][64:65, :])

        def norm_h(h, zi):
            """Per-head normalization: OT[rb(h), ct(h)] *= 1/Z[h] (after evict).
            The last head's norm is the out-projection's gating dep: run its
            mul on the (idle-at-tail) DVE and its zrep via HWDGE."""
            ct = h // 2
            rb = (h % 2) * 64
            last = h == HC - 1
            nc.sync.dma_start(out=_ap(z_dram, h * CUR, [[CUR, 1], [1, CUR]]),
                              in_=zi)
            zrep = zrpool.tile([128, CUR], F32, tag="zrep")
            (nc.sync if last else nc.gpsimd).dma_start(
                out=zrep[rb:rb + 64, :],
                in_=_ap(z_dram, h * CUR, [[0, 64], [1, CUR]]),
            )
            (nc.vector if last else nc.gpsimd).tensor_mul(
                OTs[ct][rb:rb + 64, :],
                OTs[ct][rb:rb + 64, :],
                zrep[rb:rb + 64, :],
            )

        # ---------------- emission schedule ----------------
        # Q projection; QvT written first so position stripes can start early.
        # Wq + first x chunk lead the DMA queue; small loads follow.
        wq = load_w("wA", Wq)
        for c in range(4):
            xch = x_chunk(XcT, CUR, c)
            if c == 0:
                load_small()
            for ct in range(4):
                s = slice(ct * CUR + c * 256, ct * CUR + c * 256 + 256)
                proj_chunk(wq, xch, ct, [(QvT[:, s], 1), (QuT[:, s], 0)])

        # R projection (ct0 first within each chunk)
        wpos = load_w("wB", Wpos)
        for c in range(8):
            xch = x_chunk(PosT, FULL, c)
            for ct in range(4):
                s = slice(ct * FULL + c * 256, ct * FULL + c * 256 + 256)
                proj_chunk(wpos, xch, ct, [(RT[:, s], 3)])

        # K + V projections (8 xf chunks) with head 0's position stripes
        # front-loaded, then head 0's attention paced against the remaining
        # chunks so the PE stream stays dense.
        gt0 = gtpool.tile([128, 16 * 8 * 128], BF, tag="GT")
        gts = {0: gt0}
        wk = load_w("wA", Wk)
        wv = load_w("wB", Wv)

        LAG = 2

        zis = {}

        def emit_head(h, g2=None):
            if h + 1 < HC and g2 is None:
                gt_next = gtpool.tile([128, 16 * 8 * 128], BF, tag="GT")
                gts[h + 1] = gt_next
                g2 = gen_pos(h + 1, gt_next)
            av0 = av_pool.tile([65, 512], F32, tag="av")
            av1 = av_pool.tile([65, 512], F32, tag="av")
            avs = (av0, av1)
            zi = stpool.tile([1, CUR], F32, tag="zi")
            zis[h] = zi
            pend = []
            for t in range(16):
                adv(g2, (5 if h == 6 else 4) if t < 7 else 0)
                pend.append((t, score_part(h, t, gts[h])))
                if len(pend) > LAG:
                    tp, Ep = pend.pop(0)
                    av_part(h, tp, Ep, avs)
                    if tp == 11:
                        evict_c(h, avs, 0, zi)   # av0 complete; spread burst
                yield
            for tp, Ep in pend:
                av_part(h, tp, Ep, avs)
                if tp == 11:
                    evict_c(h, avs, 0, zi)
            evict_c(h, avs, 1, zi)
            gts.pop(h)

        # K/V projections (phase 0, PE-dense on their own) with head 0's and
        # head 1's position stripes interleaved once RT is complete.
        g0 = gen_pos(0, gts[0])
        gt1 = gtpool.tile([128, 16 * 8 * 128], BF, tag="GT")
        gts[1] = gt1
        g1 = gen_pos(1, gt1)
        n2a0 = [4, 4, 4, 4, 3, 3, 3, 3]
        n2a1 = [0, 0, 0, 0, 2, 2, 3, 3]
        for c in range(8):
            xch = x_chunk(XfT, FULL, c)
            for ct in range(4):
                s = slice(ct * FULL + c * 256, ct * FULL + c * 256 + 256)
                proj_chunk(wk, xch, ct, [(KT[:, s], 2)])
                adv(g0, n2a0[c] // 4 + (1 if ct < n2a0[c] % 4 else 0))
                adv(g1, n2a1[c] // 4 + (1 if ct < n2a1[c] % 4 else 0))
            for tts in range(2):
                v_chunk(wv, xch, 2 * c + tts)
        # Wproj load now: wA (Wk) has no more readers after the loop above.
        Wproj_sb = wpool.tile([128, 8 * CW], BF, tag="wA")
        nc.sync.dma_start(out=_ap(Wproj_sb, 0, [[8 * CW, 128], [D, 4], [1, D]]),
                          in_=_blk(Wproj, D, 4))

        # norm_h(h-1) is emitted at t=10 of head h's loop so Pool's in-order
        # queue runs the next head's stripe memsets BEFORE the norm chain.
        for h in range(HC):
            gen = emit_head(h, g2=g1 if h == 0 else None)
            for t, _ in enumerate(gen):
                if t == 10 and h > 0:
                    norm_h(h - 1, zis.pop(h - 1))
        norm_h(HC - 1, zis.pop(HC - 1))

        # ---------------- output projection (software-pipelined wave) ----
        # Phase A: first NPRE blocks' ct0-2 partials run while the last
        # head's norm chain completes (they don't touch OT3). Phase B:
        # stream ct3 + evacuate + start later blocks' partials.
        NPRE = 6
        blocks = [(it, dc) for it in range(8) for dc in range(2)]
        pps_of = {}
        osb_of = {}

        def op_partial(b):
            it, dc = blocks[b]
            pool = ps_pool if b % 2 == 0 else gp_pool
            pps = pool.tile([128, 512], F32, tag="ps" if b % 2 == 0 else "gp")
            pps_of[b] = pps
            for ct in range(3):
                nc.tensor.matmul(
                    pps[:, 0:512],
                    OTs[ct][:, it * 128: it * 128 + 128],
                    Wproj_sb[:, ct * D + dc * 512: ct * D + dc * 512 + 512],
                    start=(ct == 0), stop=False,
                )

        def op_finish(b):
            it, dc = blocks[b]
            pps = pps_of.pop(b)
            nc.tensor.matmul(
                pps[:, 0:512],
                OTs[3][:, it * 128: it * 128 + 128],
                Wproj_sb[:, 3 * D + dc * 512: 3 * D + dc * 512 + 512],
                start=False, stop=True,
            )
            if dc == 0:
                osb_w = obpool.tile([128, 1024], BF, tag="osb")
                osb_of[it] = osb_w
            osb = osb_of[it]
            if b % 2 == 0:
                nc.vector.tensor_copy(osb[:, dc * 512:(dc + 1) * 512], pps[:, 0:512])
            else:
                nc.scalar.activation(osb[:, dc * 512:(dc + 1) * 512], pps[:, 0:512], CPY)
            if dc == 1:
                nc.sync.dma_start(
                    out=out_d[it * 128:(it + 1) * 128, :],
                    in_=osb_of.pop(it),
                )

        for b in range(NPRE):
            op_partial(b)
        for b in range(16):
            op_finish(b)
            if b + NPRE < 16:
                op_partial(b + NPRE)

    nc.compile()
    return nc


def _prep_core_inputs(inputs, pos_embedding, full_input, u, v,
                      W_kv, b_kv, W_q, b_q, W_pos, b_pos, W_proj):
    """Host-side shard prep: returns list of 8 in_maps."""
    bf = BF_NP
    posT = np.ascontiguousarray(pos_embedding[:, 0, :].T).astype(bf)
    ident = np.eye(128, dtype=np.float32).astype(bf)
    in_maps = []
    for c in range(8):
        b, hg = c // 2, c % 2
        s = slice(hg * CW, (hg + 1) * CW)
        hs = slice(hg * HC, (hg + 1) * HC)
        in_maps.append({
            "XcT": np.ascontiguousarray(inputs[:, b, :].T).astype(bf),
            "XfT": np.ascontiguousarray(full_input[:, b, :].T).astype(bf),
            "PosT": posT,
            "Wq": np.ascontiguousarray(W_q[:, s]).astype(bf),
            "Wk": np.ascontiguousarray(W_kv[:, :HN * HD][:, s]).astype(bf),
            "Wv": np.ascontiguousarray(W_kv[:, HN * HD:][:, s]).astype(bf),
            "Wpos": np.ascontiguousarray(W_pos[:, s]).astype(bf),
            "Wproj": np.ascontiguousarray(W_proj[s, :]).astype(bf),
            "Ident": ident,
            "qu_b": (b_q[s] + u[hs].reshape(-1)).astype(np.float32).reshape(CW, 1),
            "qv_b": (b_q[s] + v[hs].reshape(-1)).astype(np.float32).reshape(CW, 1),
            "k_b": b_kv[:HN * HD][s].astype(np.float32).reshape(CW, 1),
            "r_b": b_pos[s].astype(np.float32).reshape(CW, 1),
        })
    return in_maps


def kernel(inputs, pos_embedding, full_input, u, v, mask,
           W_kv, b_kv, W_q, b_q, W_pos, b_pos, W_proj, b_proj,
           _want_profile=False):
    inputs = np.asarray(inputs, np.float32)
    pos_embedding = np.asarray(pos_embedding, np.float32)
    full_input = np.asarray(full_input, np.float32)

    if "nc" not in _CACHE:
        _CACHE["nc"] = build_program()
    nc = _CACHE["nc"]

    in_maps = _prep_core_inputs(
        inputs, pos_embedding, full_input,
        np.asarray(u, np.float32), np.asarray(v, np.float32),
        np.asarray(W_kv, np.float32), np.asarray(b_kv, np.float32),
        np.asarray(W_q, np.float32), np.asarray(b_q, np.float32),
        np.asarray(W_pos, np.float32), np.asarray(b_pos, np.float32),
        np.asarray(W_proj, np.float32))

    res = run_bass_kernel_spmd(nc, in_maps, list(range(8)))

    b_v = np.asarray(b_kv, np.float32)[HN * HD:]
    beta = b_v @ np.asarray(W_proj, np.float32) + np.asarray(b_proj, np.float32)
    out = np.empty((CUR, BS, D), np.float32)
    for b in range(BS):
        out[:, b, :] = (res.results[2 * b]["out_part"].astype(np.float32)
                        + res.results[2 * b + 1]["out_part"].astype(np.float32)
                        + beta)
    if _want_profile:
        return out, res
    return out


# revision 91
# speedup vs baseline: 1.0099x; 1.0099x over previous
"""Transformer-XL attention on 8 Trainium2 NeuronCores (Bass/Tile).

Sharding: 8 cores = 4 batches x 2 head-groups of 8 heads.
Each core computes its (batch, head-group) attention output projected through
its W_proj row-slice; host sums the two head-group partials per batch and adds
the bias terms (b_v @ W_proj + b_proj) once.

v2 dataflow (vs v1): position logits are merged into the content logits in
PSUM via an identity-matmul accumulate on the PE, so only ONE exp per score
element runs on the Act engine (v1 ran two).  The rel-shift skew DMA stages
through a DRAM scratch ring; PSUM->SBUF conversion of position logits rides
on the DVE (last stripe on Act).  1/Z comes from a reciprocal read straight
off the PSUM z-row; per-head norms are emitted one head late so Pool's
in-order queue never blocks the next head's stripe pipeline; the output
projection is software-pipelined so its ct0-2 partials overlap the final
norm chain.

Self-contained: only imports numpy/ml_dtypes and the installed concourse stack.
"""

import sys

for _p in ("/opt/trn_rl_repo",):
    if _p not in sys.path:
        sys.path.insert(0, _p)

from contextlib import ExitStack

import ml_dtypes
import numpy as np

import concourse.bacc as bacc
import concourse.bass as bass
import concourse.mybir as mybir
import concourse.tile as tile
from concourse.bass_utils import run_bass_kernel_spmd

CUR, FULL, BS, D = 1024, 2048, 4, 1024
HN, HD = 16, 64
PREV = FULL - CUR
SCALE = 1.0 / HD**0.5
HC = 8          # heads per core
CW = HC * HD    # 512 channel columns per core
BF = mybir.dt.bfloat16
F32 = mybir.dt.float32
EXP = mybir.ActivationFunctionType.Exp
BF_NP = ml_dtypes.bfloat16
NEG = -1.0e9    # logit pad for masked region

_CACHE = {}


def _ap(t, off, dims):
    return bass.AP(tensor=t.tensor, offset=t.offset + off, ap=dims)


def _blk(d, rowlen, nblk):
    """DRAM [nblk*128, rowlen] viewed as [p, blk, col]."""
    return _ap(d, 0, [[rowlen, 128], [128 * rowlen, nblk], [1, rowlen]])


def build_program():
    nc = bacc.Bacc("TRN2", target_bir_lowering=False, debug=False)

    XcT = nc.dram_tensor("XcT", [D, CUR], BF, kind="ExternalInput").ap()
    XfT = nc.dram_tensor("XfT", [D, FULL], BF, kind="ExternalInput").ap()
    PosT = nc.dram_tensor("PosT", [D, FULL], BF, kind="ExternalInput").ap()
    Wq = nc.dram_tensor("Wq", [D, CW], BF, kind="ExternalInput").ap()
    Wk = nc.dram_tensor("Wk", [D, CW], BF, kind="ExternalInput").ap()
    Wv = nc.dram_tensor("Wv", [D, CW], BF, kind="ExternalInput").ap()
    Wpos = nc.dram_tensor("Wpos", [D, CW], BF, kind="ExternalInput").ap()
    Wproj = nc.dram_tensor("Wproj", [CW, D], BF, kind="ExternalInput").ap()
    Ident = nc.dram_tensor("Ident", [128, 128], BF, kind="ExternalInput").ap()
    qu_b_d = nc.dram_tensor("qu_b", [CW, 1], F32, kind="ExternalInput").ap()
    qv_b_d = nc.dram_tensor("qv_b", [CW, 1], F32, kind="ExternalInput").ap()
    k_b_d = nc.dram_tensor("k_b", [CW, 1], F32, kind="ExternalInput").ap()
    r_b_d = nc.dram_tensor("r_b", [CW, 1], F32, kind="ExternalInput").ap()
    out_d = nc.dram_tensor("out_part", [CUR, D], BF, kind="ExternalOutput").ap()
    z_dram = nc.dram_tensor("z_scratch", [HC, CUR], F32).ap()
    gs_dram = nc.dram_tensor("gs_scratch", [8 * 128, 2048], BF).ap()

    with tile.TileContext(nc) as tc, ExitStack() as ctx:
        persist = ctx.enter_context(tc.tile_pool(name="persist", bufs=1))
        ps_pool = ctx.enter_context(tc.tile_pool(name="ps", bufs=3, space="PSUM"))
        gp_pool = ctx.enter_context(tc.tile_pool(name="gp", bufs=3, space="PSUM"))
        av_pool = ctx.enter_context(tc.tile_pool(name="avps", bufs=2, space="PSUM"))
        gtpool = ctx.enter_context(tc.tile_pool(name="gt", bufs=2))
        gpool = ctx.enter_context(tc.tile_pool(name="g", bufs=3))
        epool = ctx.enter_context(tc.tile_pool(name="e", bufs=4))
        zrpool = ctx.enter_context(tc.tile_pool(name="zr", bufs=1))
        stpool = ctx.enter_context(tc.tile_pool(name="st", bufs=2))
        obpool = ctx.enter_context(tc.tile_pool(name="ob", bufs=3))
        wpool = ctx.enter_context(tc.tile_pool(name="wp", bufs=1))
        xpool = ctx.enter_context(tc.tile_pool(name="xp", bufs=3))

        QuT = persist.tile([128, 4 * CUR], BF, tag="QuT")
        QvT = persist.tile([128, 4 * CUR], BF, tag="QvT")
        KT = persist.tile([128, 4 * FULL], BF, tag="KT")
        RT = persist.tile([128, 4 * FULL], BF, tag="RT")
        Vp = persist.tile([128, 16 * 8 * 66], BF, tag="Vp")
        OT0 = persist.tile([128, CUR], BF, tag="OT0")
        OT1 = persist.tile([128, CUR], BF, tag="OT1")
        OT2 = persist.tile([128, CUR], BF, tag="OT2")
        OT3 = persist.tile([128, CUR], BF, tag="OT3")
        OTs = [OT0, OT1, OT2, OT3]
        Id = persist.tile([128, 128], BF, tag="Id")
        biases = persist.tile([128, 16], F32, tag="biases")
        CPY = mybir.ActivationFunctionType.Copy

        def load_small():
            for bi, bd in enumerate((qu_b_d, qv_b_d, k_b_d, r_b_d)):
                nc.sync.dma_start(
                    out=biases[:, bi * 4:(bi + 1) * 4],
                    in_=_ap(bd, 0, [[1, 128], [128, 4]]),
                )
            nc.sync.dma_start(out=Id, in_=Ident)
            # ones columns of V' (col 64 of each 66-wide head slot)
            nc.vector.memset(
                _ap(Vp, 64, [[16 * 8 * 66, 128], [8 * 66, 16], [66, 8], [1, 1]]), 1.0)

        # ---------------- streamed projections ----------------
        def load_w(tag, wd):
            w = wpool.tile([128, 8 * CW], BF, tag=tag)
            with tc.high_priority():
                nc.sync.dma_start(out=w.rearrange("p (kt c) -> p kt c", kt=8),
                                  in_=_blk(wd, CW, 8))
            return w

        def x_chunk(src, n_total, c):
            """256-token chunk c of DRAM [D, n_total] -> [128, 8kt x 256]."""
            xch = xpool.tile([128, 2048], BF, tag="xch")
            with tc.high_priority():
                nc.sync.dma_start(
                    out=xch.rearrange("p (k c) -> p k c", k=8),
                    in_=_ap(src, c * 256, [[n_total, 128], [128 * n_total, 8], [1, 256]]))
            return xch

        def proj_chunk(w_sb, xch, ct, dests):
            """dests: list of (out_256col_ap, bias_col)."""
            ps = ps_pool.tile([128, 512], F32, tag="ps")
            for kt in range(8):
                nc.tensor.matmul(
                    ps[:, 0:256],
                    w_sb[:, kt * CW + ct * 128: kt * CW + ct * 128 + 128],
                    xch[:, kt * 256: kt * 256 + 256],
                    start=(kt == 0), stop=(kt == 7),
                )
            for dest, bcol in dests:
                nc.vector.tensor_scalar(
                    dest, ps[:, 0:256],
                    biases[:, bcol * 4 + ct: bcol * 4 + ct + 1],
                    None, mybir.AluOpType.add,
                )

        def v_chunk(wv_sb, xch, tt):
            """V projection for 128-token block tt (tokens tt*128..+128)."""
            tts = tt % 2
            ps = ps_pool.tile([128, 512], F32, tag="ps")
            for kt in range(8):
                nc.tensor.matmul(
                    ps[:, 0:512],
                    xch[:, kt * 256 + tts * 128: kt * 256 + tts * 128 + 128],
                    wv_sb[:, kt * CW: kt * CW + CW],
                    start=(kt == 0), stop=(kt == 7),
                )
            nc.scalar.activation(
                _ap(Vp, tt * 8 * 66, [[16 * 8 * 66, 128], [66, 8], [1, 64]]),
                ps[:, 0:512].rearrange("p (h d) -> p h d", h=8),
                mybir.ActivationFunctionType.Copy,
            )

        # ---------------- attention helpers ----------------
        def gen_pos(h, GT):
            """Emit head h's position-logit stripes (one per query block qt).
            Each stripe: PE matmuls (512-col chunks) -> DVE convert to bf16
            Gl -> pad memset -> skew DMA -> DmaTransposeAnt into key-major GT.
            Generator yields after each 512-col CHUNK so the caller can
            interleave stripe work finely with other PE work (keeps the DVE
            conversion from falling behind the PSUM ring)."""
            ct = h // 2
            rb = (h % 2) * 64
            for qt in range(8):
                i0 = qt * 128
                m_lo = 896 - i0
                W = FULL - m_lo            # 1152 + i0 == Wj (multiple of 128)
                nblk = qt + 9
                Gl = gpool.tile([128, 2176], BF, tag="Gl")
                nc.gpsimd.memset(Gl[:, W:W + 128], NEG)
                off = 0
                while off < W:
                    wn = min(512, W - off)
                    gps = gp_pool.tile([128, 512], F32, tag="gp")
                    nc.tensor.matmul(
                        gps[:, 0:wn],
                        QvT[rb:rb + 64, ct * CUR + i0: ct * CUR + i0 + 128],
                        RT[rb:rb + 64, ct * FULL + m_lo + off:
                           ct * FULL + m_lo + off + wn],
                        start=True, stop=True,
                    )
                    if qt == 7:
                        # last stripe converts on Act: DVE is the convP
                        # pipeline's slowest stage; shed its backlog peak
                        nc.scalar.activation(Gl[:, off:off + wn], gps[:, 0:wn], CPY)
                    else:
                        nc.vector.tensor_copy(Gl[:, off:off + wn], gps[:, 0:wn])
                    off += wn
                    if off < W:
                        yield
                # skew: gs[p, j] = Gl[p, 127 + j - p]  (diagonal read) into a
                # DRAM scratch ring (deep buffering, frees SBUF)
                gs = _ap(gs_dram, ((h * 8 + qt) % 8) * 128 * 2048,
                         [[2048, 128], [1, W]])
                nc.sync.dma_start(out=gs, in_=_ap(Gl, 127, [[2176 - 1, 128], [1, W]]))
                # transpose into GT[jj, qt, t, p] = gs[p, t*128 + jj]
                nc.sync.dma_start_transpose(
                    out=_ap(GT, qt * 2048, [[16 * 8 * 128, 128], [128, nblk], [1, 128]]),
                    in_=gs,
                )
                yield

        def adv(gen, n):
            if gen is None:
                return
            for _ in range(n):
                next(gen, None)

        def score_part(h, t, GT):
            """content scores + position logits (identity-matmul accumulate)
            -> one exp on Act -> E(t).  AV is emitted LAG iterations later."""
            ct = h // 2
            rb = (h % 2) * 64
            qt_min = max(0, t - 8)
            ioff = qt_min * 128
            w = CUR - ioff
            E = epool.tile([128, 1024], BF, tag="E")
            sc = 0
            while sc < w:
                wn = min(512, w - sc)
                nqt = wn // 128
                cps = ps_pool.tile([128, 512], F32, tag="ps")
                nc.tensor.matmul(
                    cps[:, 0:wn],
                    KT[rb:rb + 64, ct * FULL + t * 128: ct * FULL + t * 128 + 128],
                    QuT[rb:rb + 64, ct * CUR + ioff + sc: ct * CUR + ioff + sc + wn],
                    start=True, stop=False,
                )
                nc.tensor.matmul(
                    cps[:, 0:wn],
                    Id,
                    _ap(GT, (qt_min + sc // 128) * 2048 + t * 128,
                        [[16 * 8 * 128, 128], [2048, nqt], [1, 128]]),
                    start=False, stop=True,
                )
                nc.scalar.activation(E[:, sc:sc + wn], cps[:, 0:wn], EXP, scale=SCALE)
                sc += wn
            return E

        def av_part(h, t, E, avs):
            ioff = max(0, t - 8) * 128
            for c in range(2):
                lo = max(ioff, c * 512)
                hi = (c + 1) * 512
                if lo >= hi:
                    continue
                last_t = 11 if c == 0 else 15
                nc.tensor.matmul(
                    avs[c][:, lo - c * 512: hi - c * 512],
                    Vp[:, t * 8 * 66 + h * 66: t * 8 * 66 + h * 66 + 65],
                    E[:, lo - ioff: hi - ioff],
                    start=(t == 0), stop=(t == last_t),
                )

        def evict_c(h, avs, c, zi):
            """OT eviction on Act + 1/Z straight off the PSUM z-row on DVE."""
            ct = h // 2
            rb = (h % 2) * 64
            nc.scalar.activation(
                OTs[ct][rb:rb + 64, c * 512: c * 512 + 512],
                avs[c][0:64, :], CPY,
            )
            nc.vector.reciprocal(zi[0:1, c * 512:(c + 1) * 512], avs[c][64:65, :])

        def norm_h(h, zi):
            """Per-head normalization: OT[rb(h), ct(h)] *= 1/Z[h] (after evict).
            The last head's norm is the out-projection's gating dep: run its
            mul on the (idle-at-tail) DVE and its zrep via HWDGE."""
            ct = h // 2
            rb = (h % 2) * 64
            last = h == HC - 1
            nc.sync.dma_start(out=_ap(z_dram, h * CUR, [[CUR, 1], [1, CUR]]),
                              in_=zi)
            zrep = zrpool.tile([128, CUR], F32, tag="zrep")
            (nc.sync if last else nc.gpsimd).dma_start(
                out=zrep[rb:rb + 64, :],
                in_=_ap(z_dram, h * CUR, [[0, 64], [1, CUR]]),
            )
            (nc.vector if last else nc.gpsimd).tensor_mul(
                OTs[ct][rb:rb + 64, :],
                OTs[ct][rb:rb + 64, :],
                zrep[rb:rb + 64, :],
            )

        # ---------------- emission schedule ----------------
        # Q projection; QvT written first so position stripes can start early.
        # Wq + first x chunk lead the DMA queue; small loads follow.
        wq = load_w("wA", Wq)
        for c in range(4):
            xch = x_chunk(XcT, CUR, c)
            if c == 0:
                load_small()
            for ct in range(4):
                s = slice(ct * CUR + c * 256, ct * CUR + c * 256 + 256)
                proj_chunk(wq, xch, ct, [(QvT[:, s], 1), (QuT[:, s], 0)])

        # R projection (ct0 first within each chunk)
        wpos = load_w("wB", Wpos)
        for c in range(8):
            xch = x_chunk(PosT, FULL, c)
            for ct in range(4):
                s = slice(ct * FULL + c * 256, ct * FULL + c * 256 + 256)
                proj_chunk(wpos, xch, ct, [(RT[:, s], 3)])

        # K + V projections (8 xf chunks) with head 0's position stripes
        # front-loaded, then head 0's attention paced against the remaining
        # chunks so the PE stream stays dense.
        gt0 = gtpool.tile([128, 16 * 8 * 128], BF, tag="GT")
        gts = {0: gt0}
        wk = load_w("wA", Wk)
        wv = load_w("wB", Wv)

        LAG = 3

        zis = {}

        def emit_head(h, g2=None):
            if h + 1 < HC and g2 is None:
                gt_next = gtpool.tile([128, 16 * 8 * 128], BF, tag="GT")
                gts[h + 1] = gt_next
                g2 = gen_pos(h + 1, gt_next)
            av0 = av_pool.tile([65, 512], F32, tag="av")
            av1 = av_pool.tile([65, 512], F32, tag="av")
            avs = (av0, av1)
            zi = stpool.tile([1, CUR], F32, tag="zi")
            zis[h] = zi
            pend = []
            for t in range(16):
                adv(g2, (5 if h == 6 else 4) if t < 7 else 0)
                pend.append((t, score_part(h, t, gts[h])))
                if len(pend) > LAG:
                    tp, Ep = pend.pop(0)
                    av_part(h, tp, Ep, avs)
                    if tp == 11:
                        evict_c(h, avs, 0, zi)   # av0 complete; spread burst
                yield
            for tp, Ep in pend:
                av_part(h, tp, Ep, avs)
                if tp == 11:
                    evict_c(h, avs, 0, zi)
            evict_c(h, avs, 1, zi)
            gts.pop(h)

        # K/V projections (phase 0, PE-dense on their own) with head 0's and
        # head 1's position stripes interleaved once RT is complete.
        g0 = gen_pos(0, gts[0])
        gt1 = gtpool.tile([128, 16 * 8 * 128], BF, tag="GT")
        gts[1] = gt1
        g1 = gen_pos(1, gt1)
        n2a0 = [4, 4, 4, 4, 3, 3, 3, 3]
        n2a1 = [0, 0, 0, 0, 2, 2, 3, 3]
        for c in range(8):
            xch = x_chunk(XfT, FULL, c)
            for ct in range(4):
                s = slice(ct * FULL + c * 256, ct * FULL + c * 256 + 256)
                proj_chunk(wk, xch, ct, [(KT[:, s], 2)])
                adv(g0, n2a0[c] // 4 + (1 if ct < n2a0[c] % 4 else 0))
                adv(g1, n2a1[c] // 4 + (1 if ct < n2a1[c] % 4 else 0))
            for tts in range(2):
                v_chunk(wv, xch, 2 * c + tts)
        # Wproj load now: wA (Wk) has no more readers after the loop above.
        Wproj_sb = wpool.tile([128, 8 * CW], BF, tag="wA")
        nc.sync.dma_start(out=_ap(Wproj_sb, 0, [[8 * CW, 128], [D, 4], [1, D]]),
                          in_=_blk(Wproj, D, 4))

        # norm_h(h-1) is emitted at t=10 of head h's loop so Pool's in-order
        # queue runs the next head's stripe memsets BEFORE the norm chain.
        for h in range(HC):
            gen = emit_head(h, g2=g1 if h == 0 else None)
            for t, _ in enumerate(gen):
                if t == 10 and h > 0:
                    norm_h(h - 1, zis.pop(h - 1))
        norm_h(HC - 1, zis.pop(HC - 1))

        # ---------------- output projection (software-pipelined wave) ----
        # Phase A: first NPRE blocks' ct0-2 partials run while the last
        # head's norm chain completes (they don't touch OT3). Phase B:
        # stream ct3 + evacuate + start later blocks' partials.
        NPRE = 6
        blocks = [(it, dc) for it in range(8) for dc in range(2)]
        pps_of = {}
        osb_of = {}

        def op_partial(b):
            it, dc = blocks[b]
            pool = ps_pool if b % 2 == 0 else gp_pool
            pps = pool.tile([128, 512], F32, tag="ps" if b % 2 == 0 else "gp")
            pps_of[b] = pps
            for ct in range(3):
                nc.tensor.matmul(
                    pps[:, 0:512],
                    OTs[ct][:, it * 128: it * 128 + 128],
                    Wproj_sb[:, ct * D + dc * 512: ct * D + dc * 512 + 512],
                    start=(ct == 0), stop=False,
                )

        def op_finish(b):
            it, dc = blocks[b]
            pps = pps_of.pop(b)
            nc.tensor.matmul(
                pps[:, 0:512],
                OTs[3][:, it * 128: it * 128 + 128],
                Wproj_sb[:, 3 * D + dc * 512: 3 * D + dc * 512 + 512],
                start=False, stop=True,
            )
            if dc == 0:
                osb_w = obpool.tile([128, 1024], BF, tag="osb")
                osb_of[it] = osb_w
            osb = osb_of[it]
            if b % 2 == 0:
                nc.vector.tensor_copy(osb[:, dc * 512:(dc + 1) * 512], pps[:, 0:512])
            else:
                nc.scalar.activation(osb[:, dc * 512:(dc + 1) * 512], pps[:, 0:512], CPY)
            if dc == 1:
                nc.sync.dma_start(
                    out=out_d[it * 128:(it + 1) * 128, :],
                    in_=osb_of.pop(it),
                )

        for b in range(NPRE):
            op_partial(b)
        for b in range(16):
            op_finish(b)
            if b + NPRE < 16:
                op_partial(b + NPRE)

    nc.compile()
    return nc


def _prep_core_inputs(inputs, pos_embedding, full_input, u, v,
                      W_kv, b_kv, W_q, b_q, W_pos, b_pos, W_proj):
    """Host-side shard prep: returns list of 8 in_maps."""
    bf = BF_NP
    posT = np.ascontiguousarray(pos_embedding[:, 0, :].T).astype(bf)
    ident = np.eye(128, dtype=np.float32).astype(bf)
    in_maps = []
    for c in range(8):
        b, hg = c // 2, c % 2
        s = slice(hg * CW, (hg + 1) * CW)
        hs = slice(hg * HC, (hg + 1) * HC)
        in_maps.append({
            "XcT": np.ascontiguousarray(inputs[:, b, :].T).astype(bf),
            "XfT": np.ascontiguousarray(full_input[:, b, :].T).astype(bf),
            "PosT": posT,
            "Wq": np.ascontiguousarray(W_q[:, s]).astype(bf),
            "Wk": np.ascontiguousarray(W_kv[:, :HN * HD][:, s]).astype(bf),
            "Wv": np.ascontiguousarray(W_kv[:, HN * HD:][:, s]).astype(bf),
            "Wpos": np.ascontiguousarray(W_pos[:, s]).astype(bf),
            "Wproj": np.ascontiguousarray(W_proj[s, :]).astype(bf),
            "Ident": ident,
            "qu_b": (b_q[s] + u[hs].reshape(-1)).astype(np.float32).reshape(CW, 1),
            "qv_b": (b_q[s] + v[hs].reshape(-1)).astype(np.float32).reshape(CW, 1),
            "k_b": b_kv[:HN * HD][s].astype(np.float32).reshape(CW, 1),
            "r_b": b_pos[s].astype(np.float32).reshape(CW, 1),
        })
    return in_maps


def kernel(inputs, pos_embedding, full_input, u, v, mask,
           W_kv, b_kv, W_q, b_q, W_pos, b_pos, W_proj, b_proj,
           _want_profile=False):
    inputs = np.asarray(inputs, np.float32)
    pos_embedding = np.asarray(pos_embedding, np.float32)
    full_input = np.asarray(full_input, np.float32)

    if "nc" not in _CACHE:
        _CACHE["nc"] = build_program()
    nc = _CACHE["nc"]

    in_maps = _prep_core_inputs(
        inputs, pos_embedding, full_input,
        np.asarray(u, np.float32), np.asarray(v, np.float32),
        np.asarray(W_kv, np.float32), np.asarray(b_kv, np.float32),
        np.asarray(W_q, np.float32), np.asarray(b_q, np.float32),
        np.asarray(W_pos, np.float32), np.asarray(b_pos, np.float32),
        np.asarray(W_proj, np.float32))

    res = run_bass_kernel_spmd(nc, in_maps, list(range(8)))

    b_v = np.asarray(b_kv, np.float32)[HN * HD:]
    beta = b_v @ np.asarray(W_proj, np.float32) + np.asarray(b_proj, np.float32)
    out = np.empty((CUR, BS, D), np.float32)
    for b in range(BS):
        out[:, b, :] = (res.results[2 * b]["out_part"].astype(np.float32)
                        + res.results[2 * b + 1]["out_part"].astype(np.float32)
                        + beta)
    if _want_profile:
        return out, res
    return out
